# revision 1
# baseline (speedup 1.0000x reference)
"""Trainium2 Bass kernel for ContentMultiheadAttention.

Reference computation (L=512, B=32, E=1024, H=16, hd=64):
  q,k,v = x @ W{q,k,v}.T + b    (torch F.linear convention)
  split heads -> [B*H, L, 64]; q /= 8
  S = q @ k.T;  S[mask] = -1e9;  P = softmax(S)
  O = P @ v -> merge heads -> out = O @ Wo.T + bo

Strategy: data-parallel over B across 8 cores (4 graphs/core). Per graph,
scores run in S^T layout ([k, q]) so P^T (exp * binary keep-mask, exact
zeros; scores are bounded so no max-subtraction) is in SBUF with keys on
partitions. The PV matmul uses P^T chunks as the stationary operand and
V (keys on partitions, with a ones-column appended) as the moving
operand, producing O in [q, hd] layout as ap-65 matmuls — this costs
65 cycles per (q-chunk, k-chunk) instead of 512, halving PV PE time,
and the softmax denominator lands as a per-partition (per-query) scalar
so normalization is one DVE reciprocal + tensor_scalar multiply per
head-chunk (no partition broadcast). O is transposed back to [e, q]
for the out-projection by the DMA xbar engine (free on PE). The
out-projection computes out^T (e_out on partitions) so the output bias
is a per-partition ACT bias — no bias matmuls; the host transposes the
stored out^T. The emission is a cross-graph software pipeline: in-proj
of graph b+1 and out-proj of graph b-1 are woven between the attention
head pairs of graph b to keep TensorE fed (ACT owns the exp stream).
All matmuls are bf16 with fp32 PSUM accumulation; softmax math is fp32.
"""

import numpy as np
import ml_dtypes

import concourse.mybir as mybir
import concourse.tile as tile
from concourse import bacc
from concourse import bass_utils

L, B, E, H = 512, 32, 1024, 16
HD = E // H  # 64
NCORES = 8
BPC = B // NCORES  # graphs per core

BF = mybir.dt.bfloat16
F32 = mybir.dt.float32
AF = mybir.ActivationFunctionType
ALU = mybir.AluOpType

_BUILT = {}


def _build_module():
    """Construct + compile the per-core Bacc program (same NEFF on all cores)."""
    nc = bacc.Bacc(None, target_bir_lowering=False, debug=False)

    # --- DRAM I/O (per core) ---
    # x*: [graph, p, ein_chunk, token] (X^T laid out for 128-partition tiles)
    xq = nc.dram_tensor("xq", [BPC, 128, 8, L], BF, kind="ExternalInput").ap()
    xk = nc.dram_tensor("xk", [BPC, 128, 8, L], BF, kind="ExternalInput").ap()
    xv = nc.dram_tensor("xv", [BPC, 128, 8, L], BF, kind="ExternalInput").ap()
    # mask^T as multiplicative binary (1=keep, 0=masked): [graph, p, kc, q]
    mneg = nc.dram_tensor("mneg", [BPC, 128, 4, L], BF, kind="ExternalInput").ap()
    # Wq/Wk^T sliced by e_out chunk for early compute start: [eo, p, ei, col]
    wq = nc.dram_tensor("wq", [8, 128, 8, 128], BF, kind="ExternalInput").ap()
    wk = nc.dram_tensor("wk", [8, 128, 8, 128], BF, kind="ExternalInput").ap()
    # Wv/Wo^T: [p, ein_chunk, e_out] (Wo chunked pair-major to match oat)
    wv = nc.dram_tensor("wv", [128, 8, E], BF, kind="ExternalInput").ap()
    wo = nc.dram_tensor("wo", [128, 8, E], BF, kind="ExternalInput").ap()
    # q/k biases per e_out partition: [p, eo_chunk]
    bq = nc.dram_tensor("bq", [128, 8], F32, kind="ExternalInput").ap()
    bk = nc.dram_tensor("bk", [128, 8], F32, kind="ExternalInput").ap()
    # effective output bias (bo + Wo @ bv) per e_out partition: [p, eoc]
    bo2 = nc.dram_tensor("bo2", [128, 8], F32, kind="ExternalInput").ap()
    # out^T tiles: [graph, eo_chunk, p, token]; host transposes back.
    # bf16 keeps the final copy single-wait (8 dedicated f_sb buffers fit)
    # and halves store traffic; host converts to fp32.
    out = nc.dram_tensor("out", [BPC, 8, 128, L], BF, kind="ExternalOutput").ap()

    with tile.TileContext(nc) as tc:
        with (
            tc.tile_pool(name="wpool", bufs=1) as wpool,
            tc.tile_pool(name="xpool", bufs=1) as xpool,
            tc.tile_pool(name="gpool", bufs=2) as gpool,
            tc.tile_pool(name="spool", bufs=3) as spool,
            tc.tile_pool(name="ppsum", bufs=2, space="PSUM") as ppsum,
            tc.tile_pool(name="spsum", bufs=2, space="PSUM") as spsum,
            tc.tile_pool(name="opsum", bufs=2, space="PSUM") as opsum,
        ):
            # resident weights. wq/wk are eo-sliced [p, eo, ei, col] so the
            # first in-proj matmuls can start after one slice + half of x.
            wq_sb = wpool.tile([128, 8, 8, 128], BF)
            wk_sb = wpool.tile([128, 8, 8, 128], BF)
            wv_sb = wpool.tile([128, 8, E], BF)
            wo_sb = wpool.tile([128, 8, E], BF)
            bq_sb = wpool.tile([128, 8], F32)
            bk_sb = wpool.tile([128, 8], F32)
            bo2_sb = wpool.tile([128, 8], F32)

            def load_graph(b):
                st = {}
                st["xq"] = xpool.tile([128, 8, L], BF, tag="xq", name="xq")
                st["xk"] = xpool.tile([128, 8, L], BF, tag="xk", name="xk")
                st["xv"] = xpool.tile([128, 8, L], BF, tag="xv", name="xv")
                st["mneg"] = xpool.tile([128, 4, L], BF, tag="mneg", bufs=2, name="mneg")
                if b == 0:
                    # startup-ordered loads: interleave x slices with W
                    # eo-slices so the first QT/KT psum groups start ASAP
                    nc.sync.dma_start(st["xq"][:, 0:1, :], xq[b, :, 0:1, :])
                    nc.sync.dma_start(wq_sb[:, 0, 0:4], wq[0, :, 0:4])
                    nc.sync.dma_start(bq_sb[:], bq[:])
                    nc.sync.dma_start(st["xq"][:, 1:3, :], xq[b, :, 1:3, :])
                    nc.sync.dma_start(wq_sb[:, 0, 4:8], wq[0, :, 4:8])
                    nc.sync.dma_start(st["xq"][:, 3:8, :], xq[b, :, 3:8, :])
                    for eo in range(1, 8):
                        nc.sync.dma_start(wq_sb[:, eo], wq[eo])
                    nc.sync.dma_start(st["xk"][:, 0:4, :], xk[b, :, 0:4, :])
                    nc.sync.dma_start(wk_sb[:, 0], wk[0])
                    nc.sync.dma_start(bk_sb[:], bk[:])
                    nc.sync.dma_start(st["xk"][:, 4:8, :], xk[b, :, 4:8, :])
                    for eo in range(1, 8):
                        nc.sync.dma_start(wk_sb[:, eo], wk[eo])
                    nc.sync.dma_start(st["xv"][:], xv[b])
                    nc.sync.dma_start(wv_sb[:], wv[:])
                    nc.sync.dma_start(st["mneg"][:], mneg[b])
                    nc.sync.dma_start(wo_sb[:], wo[:])
                    nc.sync.dma_start(bo2_sb[:], bo2[:])
                else:
                    nc.sync.dma_start(st["xq"][:], xq[b])
                    nc.sync.dma_start(st["xk"][:], xk[b])
                    nc.sync.dma_start(st["xv"][:], xv[b])
                    nc.sync.dma_start(st["mneg"][:], mneg[b])
                st["qt"] = gpool.tile([128, 8, L], BF, tag="qt", name="qt")
                st["kt"] = gpool.tile([128, 8, L], BF, tag="kt", name="kt")
                st["vx"] = gpool.tile([128, 4, H, HD + 1], BF, tag="vx", name="vx")
                st["osb"] = gpool.tile([128, 4, E], BF, tag="osb", bufs=1,
                                       name="osb")
                st["oat"] = gpool.tile([128, 8, L], BF, tag="oat", bufs=4,
                                       name="oat")
                nc.vector.memset(st["vx"][:, :, :, HD], 1.0)
                return st

            def inproj_pieces(st):
                """24 emit-closures: 16 QT/KT psum groups + 8 V groups."""
                pieces = []
                for w_sb, xkey, dkey, bias_sb in (
                    (wq_sb, "xq", "qt", bq_sb),
                    (wk_sb, "xk", "kt", bk_sb),
                ):
                    for eo in range(8):
                        def qk_piece(w_sb=w_sb, xkey=xkey, dkey=dkey,
                                     bias_sb=bias_sb, eo=eo):
                            ps = ppsum.tile([128, 512], F32, tag="ppsum")
                            for ei in range(8):
                                nc.tensor.matmul(
                                    ps[:],
                                    w_sb[:, eo, ei, :],
                                    st[xkey][:, ei, :],
                                    start=(ei == 0),
                                    stop=(ei == 7),
                                )
                            nc.scalar.activation(
                                st[dkey][:, eo, :], ps[:], AF.Identity,
                                bias=bias_sb[:, eo : eo + 1], scale=1.0,
                            )
                        pieces.append(qk_piece)
                for t4 in range(4):
                    for ec in range(2):
                        def v_piece(t4=t4, ec=ec):
                            ps = ppsum.tile([128, 512], F32, tag="ppsum")
                            for ei in range(8):
                                nc.tensor.matmul(
                                    ps[:],
                                    st["xv"][:, ei, t4 * 128 : (t4 + 1) * 128],
                                    wv_sb[:, ei, ec * 512 : (ec + 1) * 512],
                                    start=(ei == 0),
                                    stop=(ei == 7),
                                )
                            nc.scalar.activation(
                                st["vx"][:, t4, ec * 8 : (ec + 1) * 8, 0:HD],
                                ps.rearrange("p (h d) -> p h d", d=HD),
                                AF.Copy,
                            )
                        pieces.append(v_piece)
                return pieces

            def emit_scores(st, hp, pts=None, upto=4):
                """Score pair (par0|par1) lands in one 2-bank psum tile so a
                single fused [128,1024] exp covers both heads of the pair.
                Emitted in two halves (kc<2, kc>=2) so other PE work can sit
                between them while exp frees the psum pool."""
                if pts is None:
                    pts = []
                for kc in range(len(pts), upto):
                    sps = spsum.tile([128, 2, 512], F32, tag="spsum",
                                     padded_shape=[128, 2, 512])
                    for par in (0, 1):
                        po = par * 64
                        nc.tensor.matmul(
                            sps[:, par, :],
                            st["kt"][po : po + 64, hp, kc * 128 : (kc + 1) * 128],
                            st["qt"][po : po + 64, hp, :],
                            start=True,
                            stop=True,
                        )
                    pt = spool.tile([128, 2, 512], BF, tag="pt", bufs=8)
                    nc.scalar.activation(pt[:], sps[:], AF.Exp)
                    # zero masked entries (bf16 SBUF multiply)
                    for par in (0, 1):
                        nc.vector.tensor_tensor(
                            pt[:, par, :], pt[:, par, :], st["mneg"][:, kc, :],
                            op=ALU.mult,
                        )
                    pts.append(pt)
                return pts


            def emit_pv_mms(st, hp, pts):
                """P^T chunks stationary, V(+ones) moving -> O [q, hd(+1)]."""
                tiles = []
                for par in (0, 1):
                    h = 2 * hp + par
                    ops = opsum.tile([128, 4 * (HD + 1)], F32, tag="opsum",
                                     padded_shape=[128, 512])
                    for qc in range(4):
                        sl = slice(qc * 65, qc * 65 + 65)
                        for kc in range(4):
                            nc.tensor.matmul(
                                ops[:, sl],
                                pts[kc][:, par, qc * 128 : (qc + 1) * 128],
                                st["vx"][:, kc, h, :],
                                start=(kc == 0),
                                stop=(kc == 3),
                            )
                    tiles.append(ops)
                return tiles

            def emit_pv_norm(st, hp, tiles, last=False):
                """Denominator is psum column HD of each 65-block; normalize
                is a per-partition reciprocal + tensor_scalar into osb.
                Emitted after the next pair's mask multiplies so the DVE mask
                path is never queued behind the PV-dependent normalize."""
                for par in (0, 1):
                    h = 2 * hp + par
                    ops = tiles[par]
                    rcp = spool.tile([128, 4], F32, tag="rcp", bufs=4)
                    denoms = ops.rearrange("p (qc u) -> p qc u", u=65)[:, :, HD]
                    nc.vector.reciprocal(rcp[:], denoms)
                    for qc in range(4):
                        nc.vector.tensor_scalar_mul(
                            st["osb"][:, qc, h * HD : (h + 1) * HD],
                            ops[:, qc * 65 : qc * 65 + HD],
                            rcp[:, qc : qc + 1],
                        )
                # O [q, e]-range -> oat [e, q] via DMA xbar transpose; out
                # chunk c of [128, C, 128] holds transposed rows e = c*128+p
                # (pair-major, matches oat). Pairs 0-3 go after hp 3, pairs
                # 4-7 after hp 7.
                spans = {3: (0, 4), 7: (4, 8)}
                if hp in spans:
                    p0, p1 = spans[hp]
                    for qc in range(4):
                        nc.sync.dma_start(
                            st["oat"][:, p0:p1, qc * 128 : (qc + 1) * 128],
                            st["osb"][:, qc, p0 * 128 : p1 * 128],
                            transpose=True,
                        )

            def emit_attention(st, pieces, last=False):
                """Head pairs, PV one pair behind scores, in-proj pieces of
                the NEXT graph woven between pairs to keep PE fed while ACT
                runs the exp stream."""
                # On the last graph, hold back a few pieces to run after the
                # final PV so PE stays busy while the last transposes drain.
                reserve = 3 if last else 0
                prev = None
                for hp in range(8):
                    pts = emit_scores(st, hp, upto=2)
                    if prev is not None:
                        # PV matmuls of the previous pair sit between score
                        # kc chunks so PE has ready work while the exp stream
                        # frees the 2-buffer score psum pool (PE is in-order).
                        tiles = emit_pv_mms(st, hp - 1, prev)
                    emit_scores(st, hp, pts=pts, upto=4)
                    if prev is not None:
                        emit_pv_norm(st, hp - 1, tiles, last=last)
                    n_pop = (len(pieces) - reserve + (7 - hp)) // (8 - hp)
                    for _ in range(max(0, n_pop)):
                        if len(pieces) > reserve:
                            pieces.pop(0)()
                    prev = pts
                tiles = emit_pv_mms(st, 7, prev)
                emit_pv_norm(st, 7, tiles, last=last)
                while pieces:
                    pieces.pop(0)()

            def outproj_pieces(st, b, final=False):
                """out^T tiles [e_out chunk, tokens]; bias is per-partition.
                The very last piece splits its copy+store into chunks so the
                final DMA chain drains sooner after the last matmul."""
                pieces = []
                for eoc in range(8):
                    def o_piece(eoc=eoc):
                        fps = ppsum.tile([128, 512], F32, tag="ppsum")
                        f_sb = spool.tile([128, 512], BF, tag="fsb", bufs=7)
                        for hp in range(8):
                            nc.tensor.matmul(
                                fps[:],
                                wo_sb[:, hp, eoc * 128 : (eoc + 1) * 128],
                                st["oat"][:, hp, :],
                                start=(hp == 0),
                                stop=(hp == 7),
                            )
                        # stores go out on the otherwise-idle GPSIMD (SWDGE)
                        # queue so their copy-waits never clog SP.SEQ, which
                        # carries the transposes and input loads.
                        if final and eoc >= 5:
                            # tail stores ride the (idle) ACT hwdge queue in
                            # natural order right behind their copies
                            nc.scalar.activation(
                                f_sb[:], fps[:], AF.Identity,
                                bias=bo2_sb[:, eoc : eoc + 1], scale=1.0,
                            )
                            nc.scalar.dma_start(out[b, eoc], f_sb[:])
                        else:
                            nc.scalar.activation(
                                f_sb[:], fps[:], AF.Identity,
                                bias=bo2_sb[:, eoc : eoc + 1], scale=1.0,
                            )
                            nc.gpsimd.dma_start(out[b, eoc], f_sb[:])
                    pieces.append(o_piece)
                return pieces

            # Weave plan (keeps every attention phase PE-bound vs the ACT
            # exp stream): att(0): inproj(1); att(1): inproj(2)+op(0)[:4];
            # att(2): inproj(3); att(3): op(0)[4:]+op(1)+op(2) (20 pieces so
            # the final attention phase stays PE-bound too; oat bufs=3).
            st = load_graph(0)
            for p in inproj_pieces(st):
                p()
            states = [st]
            deferred = []
            for b in range(1, BPC):
                st_next = load_graph(b)
                pieces = inproj_pieces(st_next)
                if b == 2:
                    deferred += outproj_pieces(states[0], 0)
                elif b == 3:
                    deferred += outproj_pieces(states[1], 1)
                emit_attention(states[b - 1], pieces)
                states.append(st_next)
            emit_attention(
                states[BPC - 1],
                deferred + outproj_pieces(states[BPC - 2], BPC - 2),
                last=True,
            )
            for p in outproj_pieces(states[BPC - 1], BPC - 1, final=True):
                p()

    nc.compile()
    return nc


def _prep_inputs(query, key, value, attn_mask, in_proj_weight, in_proj_bias,
                 out_proj_weight, out_proj_bias):
    bf16 = ml_dtypes.bfloat16

    def xt_layout(x):  # [L, B, E] -> [B, 128, 8, L]
        return np.ascontiguousarray(
            x.reshape(L, B, 8, 128).transpose(1, 3, 2, 0)
        ).astype(bf16)

    def wt_layout(w):  # [e_out, e_in] -> W^T as [128, 8, e_out]
        return np.ascontiguousarray(
            w.T.reshape(8, 128, E).transpose(1, 0, 2)
        ).astype(bf16)

    def wt_eo_layout(w):  # [e_out, e_in] -> W^T as [eo, 128, 8, 128]
        # wt[p, ei, eo*128 + c] -> arr[eo, p, ei, c]
        wt = w.T.reshape(8, 128, 8, 128)  # [ei, p, eo, c]
        return np.ascontiguousarray(wt.transpose(2, 1, 0, 3)).astype(bf16)

    Wq = in_proj_weight[0:E] / np.float32(np.sqrt(HD))
    Wk = in_proj_weight[E : 2 * E]
    Wv = in_proj_weight[2 * E : 3 * E]
    bq_e = in_proj_bias[0:E] / np.float32(np.sqrt(HD))
    bk_e = in_proj_bias[E : 2 * E]
    bv_e = in_proj_bias[2 * E : 3 * E]

    mneg = np.where(attn_mask, np.float32(0.0), np.float32(1.0))  # [B, q, k]
    # -> [B, k, q] -> [B, 128, 4, q]
    mneg = np.ascontiguousarray(
        mneg.transpose(0, 2, 1).reshape(B, 4, 128, L).transpose(0, 2, 1, 3)
    ).astype(bf16)

    effb = (out_proj_bias + out_proj_weight @ bv_e).astype(np.float32)

    host = {
        "xq": xt_layout(query),
        "xk": xt_layout(key),
        "xv": xt_layout(value),
        "mneg": mneg,
        "wq": wt_eo_layout(Wq),
        "wk": wt_eo_layout(Wk),
        "wv": wt_layout(Wv),
        "wo": wt_layout(out_proj_weight),
        "bq": np.ascontiguousarray(bq_e.reshape(8, 128).T).astype(np.float32),
        "bk": np.ascontiguousarray(bk_e.reshape(8, 128).T).astype(np.float32),
        "bo2": np.ascontiguousarray(effb.reshape(8, 128).T).astype(np.float32),
    }
    shared = {k: host[k] for k in ("wq", "wk", "wv", "wo", "bq", "bk", "bo2")}
    in_maps = []
    for c in range(NCORES):
        sl = slice(c * BPC, (c + 1) * BPC)
        m = dict(shared)
        m["xq"] = np.ascontiguousarray(host["xq"][sl])
        m["xk"] = np.ascontiguousarray(host["xk"][sl])
        m["xv"] = np.ascontiguousarray(host["xv"][sl])
        m["mneg"] = np.ascontiguousarray(host["mneg"][sl])
        in_maps.append(m)
    return in_maps


def kernel(query, key, value, attn_mask, in_proj_weight, in_proj_bias,
           out_proj_weight, out_proj_bias, num_heads, _trace=False):
    query = np.asarray(query, dtype=np.float32)
    key = np.asarray(key, dtype=np.float32)
    value = np.asarray(value, dtype=np.float32)
    attn_mask = np.asarray(attn_mask)
    in_proj_weight = np.asarray(in_proj_weight, dtype=np.float32)
    in_proj_bias = np.asarray(in_proj_bias, dtype=np.float32)
    out_proj_weight = np.asarray(out_proj_weight, dtype=np.float32)
    out_proj_bias = np.asarray(out_proj_bias, dtype=np.float32)
    assert int(num_heads) == H

    if "nc" not in _BUILT:
        _BUILT["nc"] = _build_module()
    nc = _BUILT["nc"]

    in_maps = _prep_inputs(query, key, value, attn_mask, in_proj_weight,
                           in_proj_bias, out_proj_weight, out_proj_bias)
    res = bass_utils.run_bass_kernel_spmd(
        nc, in_maps, core_ids=list(range(NCORES)), trace=_trace
    )
    outs = np.stack([np.asarray(r["out"], dtype=np.float32)
                     for r in res.results])  # [8, BPC, 8, 128, L]
    # full[l, c*BPC+j, eoc*128+p] = outs[c, j, eoc, p, l]
    full = outs.transpose(4, 0, 1, 2, 3).reshape(L, B, E)
    if _trace:
        return np.ascontiguousarray(full.astype(np.float32)), res
    return np.ascontiguousarray(full.astype(np.float32))



# revision 8
# speedup vs baseline: 1.1288x; 1.1288x over previous
"""Trainium2 Bass kernel for ContentMultiheadAttention.

Reference computation (L=512, B=32, E=1024, H=16, hd=64):
  q,k,v = x @ W{q,k,v}.T + b    (torch F.linear convention)
  split heads -> [B*H, L, 64]; q /= 8
  S = q @ k.T;  S[mask] = -1e9;  P = softmax(S)
  O = P @ v -> merge heads -> out = O @ Wo.T + bo

Strategy: data-parallel over B across 8 cores (4 graphs/core). Per graph,
scores run in S^T layout ([k, q]) so P^T (exp * binary keep-mask, exact
zeros; scores are bounded so no max-subtraction) is in SBUF with keys on
partitions. The PV matmul uses P^T chunks as the stationary operand and
V (keys on partitions, with a ones-column appended) as the moving
operand, producing O in [q, hd] layout as ap-65 matmuls — this costs
65 cycles per (q-chunk, k-chunk) instead of 512, halving PV PE time,
and the softmax denominator lands as a per-partition (per-query) scalar
so normalization is one DVE reciprocal + tensor_scalar multiply per
head-chunk (no partition broadcast). O is transposed back to [e, q]
for the out-projection by the DMA xbar engine (free on PE). The
out-projection computes out^T (e_out on partitions) so the output bias
is a per-partition ACT bias — no bias matmuls; the host transposes the
stored out^T. The emission is a cross-graph software pipeline: in-proj
of graph b+1 and out-proj of graph b-1 are woven between the attention
head pairs of graph b to keep TensorE fed (ACT owns the exp stream).
All matmuls are bf16 with fp32 PSUM accumulation; softmax math is fp32.
"""

import numpy as np
import ml_dtypes

import concourse.mybir as mybir
import concourse.tile as tile
from concourse import bacc
from concourse import bass_utils

L, B, E, H = 512, 32, 1024, 16
HD = E // H  # 64
NCORES = 8
BPC = B // NCORES  # graphs per core

BF = mybir.dt.bfloat16
FP8 = mybir.dt.float8e4
F32 = mybir.dt.float32
AF = mybir.ActivationFunctionType
ALU = mybir.AluOpType
DR = mybir.MatmulPerfMode.DoubleRow

# Host pre-scales for the fp8 hi/lo split (undone by ACT scale=1/(XS*WS)).
# W entries are tiny (sigma ~0.022); scaling keeps the lo residual above the
# e4m3 subnormal floor (2^-9).
XS = 4.0
WS = 32.0
INV_S = 1.0 / (XS * WS)

_BUILT = {}


def _build_module():
    """Construct + compile the per-core Bacc program (same NEFF on all cores)."""
    nc = bacc.Bacc(None, target_bir_lowering=False, debug=False)

    # --- DRAM I/O (per core) ---
    # x*: [graph, p, hi/lo, ein_chunk, token] — fp8 hi/lo pair of X^T·XS.
    # The in-proj runs as 3-term fp8 DoubleRow products (hh + lo·hi cross
    # terms); the hi/lo pair carries ~9 mantissa bits so accuracy >= bf16.
    xq = nc.dram_tensor("xq", [BPC, 128, 2, 8, L], FP8, kind="ExternalInput").ap()
    xk = nc.dram_tensor("xk", [BPC, 128, 2, 8, L], FP8, kind="ExternalInput").ap()
    xv = nc.dram_tensor("xv", [BPC, 128, 2, 8, L], FP8, kind="ExternalInput").ap()
    # mask^T as multiplicative binary (1=keep, 0=masked): [graph, p, kc, q]
    mneg = nc.dram_tensor("mneg", [BPC, 128, 4, L], BF, kind="ExternalInput").ap()
    # Wq/Wk^T·WS fp8 hi/lo, sliced by e_out chunk: [eo, p, hi/lo, ei, col]
    wq = nc.dram_tensor("wq", [8, 128, 2, 8, 128], FP8, kind="ExternalInput").ap()
    wk = nc.dram_tensor("wk", [8, 128, 2, 8, 128], FP8, kind="ExternalInput").ap()
    # Wv^T·WS fp8 hi/lo: [p, hi/lo, ein_chunk, e_out]
    wv = nc.dram_tensor("wv", [128, 2, 8, E], FP8, kind="ExternalInput").ap()
    # Wo^T: [p, ein_chunk, e_out] (chunked pair-major to match oat)
    wo = nc.dram_tensor("wo", [128, 8, E], BF, kind="ExternalInput").ap()
    # q/k biases per e_out partition: [p, eo_chunk]
    bq = nc.dram_tensor("bq", [128, 8], F32, kind="ExternalInput").ap()
    bk = nc.dram_tensor("bk", [128, 8], F32, kind="ExternalInput").ap()
    # effective output bias (bo + Wo @ bv) per e_out partition: [p, eoc]
    bo2 = nc.dram_tensor("bo2", [128, 8], F32, kind="ExternalInput").ap()
    # out^T tiles: [graph, eo_chunk, p, token]; host transposes back.
    # bf16 keeps the final copy single-wait (8 dedicated f_sb buffers fit)
    # and halves store traffic; host converts to fp32.
    out = nc.dram_tensor("out", [BPC, 8, 128, L], BF, kind="ExternalOutput").ap()

    with tile.TileContext(nc) as tc:
        with (
            tc.tile_pool(name="wpool", bufs=1) as wpool,
            tc.tile_pool(name="xpool", bufs=1) as xpool,
            tc.tile_pool(name="gpool", bufs=2) as gpool,
            tc.tile_pool(name="spool", bufs=3) as spool,
            tc.tile_pool(name="ppsum", bufs=2, space="PSUM") as ppsum,
            tc.tile_pool(name="spsum", bufs=2, space="PSUM") as spsum,
            tc.tile_pool(name="opsum", bufs=2, space="PSUM") as opsum,
        ):
            # resident weights. wq/wk are eo-sliced [p, eo, hl, ei, col] so
            # the first in-proj matmuls can start after one slice + x-hi.
            wq_sb = wpool.tile([128, 8, 2, 8, 128], FP8)
            wk_sb = wpool.tile([128, 8, 2, 8, 128], FP8)
            wv_sb = wpool.tile([128, 2, 8, E], FP8)
            wo_sb = wpool.tile([128, 8, E], BF)
            bq_sb = wpool.tile([128, 8], F32)
            bk_sb = wpool.tile([128, 8], F32)
            bo2_sb = wpool.tile([128, 8], F32)

            def load_graph(b):
                st = {}
                st["xq"] = xpool.tile([128, 2, 8, L], FP8, tag="xq", name="xq")
                st["xk"] = xpool.tile([128, 2, 8, L], FP8, tag="xk", name="xk")
                st["xv"] = xpool.tile([128, 2, 8, L], FP8, tag="xv", name="xv")
                st["mneg"] = xpool.tile([128, 4, L], BF, tag="mneg", bufs=2, name="mneg")
                if b == 0:
                    # startup-ordered loads: x-hi + W-hi slices first so the
                    # hh matmuls of the first QT psum groups start ASAP; lo
                    # parts stream while hh runs (group order is hh, lh, hl).
                    nc.sync.dma_start(st["xq"][:, 0:1, 0:2, :], xq[b, :, 0:1, 0:2, :])
                    nc.sync.dma_start(wq_sb[:, 0, 0:1], wq[0, :, 0:1])
                    nc.sync.dma_start(bq_sb[:], bq[:])
                    nc.sync.dma_start(st["xq"][:, 0:1, 2:8, :], xq[b, :, 0:1, 2:8, :])
                    nc.sync.dma_start(st["xq"][:, 1:2, :, :], xq[b, :, 1:2, :, :])
                    nc.sync.dma_start(wq_sb[:, 0, 1:2], wq[0, :, 1:2])
                    for eo in range(1, 8):
                        nc.sync.dma_start(wq_sb[:, eo], wq[eo])
                    nc.sync.dma_start(st["xk"][:, 0:1, :, :], xk[b, :, 0:1, :, :])
                    nc.sync.dma_start(wk_sb[:, 0], wk[0])
                    nc.sync.dma_start(bk_sb[:], bk[:])
                    nc.sync.dma_start(st["xk"][:, 1:2, :, :], xk[b, :, 1:2, :, :])
                    for eo in range(1, 8):
                        nc.sync.dma_start(wk_sb[:, eo], wk[eo])
                    nc.sync.dma_start(st["xv"][:], xv[b])
                    nc.sync.dma_start(wv_sb[:], wv[:])
                    nc.sync.dma_start(st["mneg"][:], mneg[b])
                    nc.sync.dma_start(wo_sb[:], wo[:])
                    nc.sync.dma_start(bo2_sb[:], bo2[:])
                else:
                    nc.sync.dma_start(st["xq"][:], xq[b])
                    nc.sync.dma_start(st["xk"][:], xk[b])
                    nc.sync.dma_start(st["xv"][:], xv[b])
                    nc.sync.dma_start(st["mneg"][:], mneg[b])
                st["qt"] = gpool.tile([128, 8, L], BF, tag="qt", name="qt")
                st["kt"] = gpool.tile([128, 8, L], BF, tag="kt", name="kt")
                st["vx"] = gpool.tile([128, 4, H, HD + 1], BF, tag="vx", name="vx")
                st["osb"] = gpool.tile([128, 4, E], BF, tag="osb", bufs=1,
                                       name="osb")
                st["oat"] = gpool.tile([128, 8, L], BF, tag="oat", bufs=4,
                                       name="oat")
                nc.vector.memset(st["vx"][:, :, :, HD], 1.0)
                return st

            # 3-term hi/lo product order: hh, lh (x-lo), hl (w-lo). Each term
            # runs as 4 fp8 DoubleRow matmuls over ei-chunk pairs (K=256 per
            # instr at 0.5 cycles/row -> 12*256 rows vs bf16's 8*512).
            HL_TERMS = ((0, 0), (0, 1), (1, 0))  # (w hi/lo, x hi/lo)

            def inproj_pieces(st):
                """24 emit-closures: 16 QT/KT psum groups + 8 V groups."""
                pieces = []
                for w_sb, xkey, dkey, bias_sb in (
                    (wq_sb, "xq", "qt", bq_sb),
                    (wk_sb, "xk", "kt", bk_sb),
                ):
                    for eo in range(8):
                        def qk_piece(w_sb=w_sb, xkey=xkey, dkey=dkey,
                                     bias_sb=bias_sb, eo=eo):
                            ps = ppsum.tile([128, 512], F32, tag="ppsum")
                            idx = 0
                            for whl, xhl in HL_TERMS:
                                for jp in range(4):
                                    nc.tensor.matmul(
                                        ps[:],
                                        w_sb[:, eo, whl, 2 * jp : 2 * jp + 2, :],
                                        st[xkey][:, xhl, 2 * jp : 2 * jp + 2, :],
                                        start=(idx == 0),
                                        stop=(idx == 11),
                                        perf_mode=DR,
                                    )
                                    idx += 1
                            nc.scalar.activation(
                                st[dkey][:, eo, :], ps[:], AF.Identity,
                                bias=bias_sb[:, eo : eo + 1], scale=INV_S,
                            )
                        pieces.append(qk_piece)
                for t4 in range(4):
                    for ec in range(2):
                        def v_piece(t4=t4, ec=ec):
                            ps = ppsum.tile([128, 512], F32, tag="ppsum")
                            idx = 0
                            for whl, xhl in HL_TERMS:
                                for jp in range(4):
                                    nc.tensor.matmul(
                                        ps[:],
                                        st["xv"][:, xhl, 2 * jp : 2 * jp + 2,
                                                 t4 * 128 : (t4 + 1) * 128],
                                        wv_sb[:, whl, 2 * jp : 2 * jp + 2,
                                              ec * 512 : (ec + 1) * 512],
                                        start=(idx == 0),
                                        stop=(idx == 11),
                                        perf_mode=DR,
                                    )
                                    idx += 1
                            nc.scalar.activation(
                                st["vx"][:, t4, ec * 8 : (ec + 1) * 8, 0:HD],
                                ps.rearrange("p (h d) -> p h d", d=HD),
                                AF.Copy, scale=INV_S,
                            )
                        pieces.append(v_piece)
                return pieces

            def emit_scores(st, hp, pts=None, upto=4):
                """Score pair (par0|par1) lands in one 2-bank psum tile so a
                single fused [128,1024] exp covers both heads of the pair.
                Emitted in two halves (kc<2, kc>=2) so other PE work can sit
                between them while exp frees the psum pool."""
                if pts is None:
                    pts = []
                for kc in range(len(pts), upto):
                    sps = spsum.tile([128, 2, 512], F32, tag="spsum",
                                     padded_shape=[128, 2, 512])
                    for par in (0, 1):
                        po = par * 64
                        nc.tensor.matmul(
                            sps[:, par, :],
                            st["kt"][po : po + 64, hp, kc * 128 : (kc + 1) * 128],
                            st["qt"][po : po + 64, hp, :],
                            start=True,
                            stop=True,
                        )
                    pt = spool.tile([128, 2, 512], BF, tag="pt", bufs=8)
                    nc.scalar.activation(pt[:], sps[:], AF.Exp)
                    # zero masked entries (bf16 SBUF multiply)
                    for par in (0, 1):
                        nc.vector.tensor_tensor(
                            pt[:, par, :], pt[:, par, :], st["mneg"][:, kc, :],
                            op=ALU.mult,
                        )
                    pts.append(pt)
                return pts


            def emit_pv_mms(st, hp, pts):
                """P^T chunks stationary, V(+ones) moving -> O [q, hd(+1)]."""
                tiles = []
                for par in (0, 1):
                    h = 2 * hp + par
                    ops = opsum.tile([128, 4 * (HD + 1)], F32, tag="opsum",
                                     padded_shape=[128, 512])
                    for qc in range(4):
                        sl = slice(qc * 65, qc * 65 + 65)
                        for kc in range(4):
                            nc.tensor.matmul(
                                ops[:, sl],
                                pts[kc][:, par, qc * 128 : (qc + 1) * 128],
                                st["vx"][:, kc, h, :],
                                start=(kc == 0),
                                stop=(kc == 3),
                            )
                    tiles.append(ops)
                return tiles

            def emit_pv_norm(st, hp, tiles, last=False):
                """Denominator is psum column HD of each 65-block; normalize
                is a per-partition reciprocal + tensor_scalar into osb.
                Emitted after the next pair's mask multiplies so the DVE mask
                path is never queued behind the PV-dependent normalize."""
                for par in (0, 1):
                    h = 2 * hp + par
                    ops = tiles[par]
                    rcp = spool.tile([128, 4], F32, tag="rcp", bufs=4)
                    denoms = ops.rearrange("p (qc u) -> p qc u", u=65)[:, :, HD]
                    nc.vector.reciprocal(rcp[:], denoms)
                    for qc in range(4):
                        nc.vector.tensor_scalar_mul(
                            st["osb"][:, qc, h * HD : (h + 1) * HD],
                            ops[:, qc * 65 : qc * 65 + HD],
                            rcp[:, qc : qc + 1],
                        )
                # O [q, e]-range -> oat [e, q] via DMA xbar transpose; out
                # chunk c of [128, C, 128] holds transposed rows e = c*128+p
                # (pair-major, matches oat). Pairs 0-3 go after hp 3, pairs
                # 4-7 after hp 7.
                spans = {3: (0, 4), 7: (4, 8)}
                if hp in spans:
                    p0, p1 = spans[hp]
                    for qc in range(4):
                        nc.sync.dma_start(
                            st["oat"][:, p0:p1, qc * 128 : (qc + 1) * 128],
                            st["osb"][:, qc, p0 * 128 : p1 * 128],
                            transpose=True,
                        )

            def emit_attention(st, pieces, last=False):
                """Head pairs, PV one pair behind scores, in-proj pieces of
                the NEXT graph woven between pairs to keep PE fed while ACT
                runs the exp stream."""
                # On the last graph, hold back a few pieces to run after the
                # final PV so PE stays busy while the last transposes drain.
                reserve = 3 if last else 0
                prev = None
                for hp in range(8):
                    pts = emit_scores(st, hp, upto=2)
                    if prev is not None:
                        # PV matmuls of the previous pair sit between score
                        # kc chunks so PE has ready work while the exp stream
                        # frees the 2-buffer score psum pool (PE is in-order).
                        tiles = emit_pv_mms(st, hp - 1, prev)
                    emit_scores(st, hp, pts=pts, upto=4)
                    if prev is not None:
                        emit_pv_norm(st, hp - 1, tiles, last=last)
                    n_pop = (len(pieces) - reserve + (7 - hp)) // (8 - hp)
                    for _ in range(max(0, n_pop)):
                        if len(pieces) > reserve:
                            pieces.pop(0)()
                    prev = pts
                tiles = emit_pv_mms(st, 7, prev)
                emit_pv_norm(st, 7, tiles, last=last)
                while pieces:
                    pieces.pop(0)()

            def outproj_pieces(st, b, final=False):
                """out^T tiles [e_out chunk, tokens]; bias is per-partition.
                The very last piece splits its copy+store into chunks so the
                final DMA chain drains sooner after the last matmul."""
                pieces = []
                for eoc in range(8):
                    def o_piece(eoc=eoc):
                        fps = ppsum.tile([128, 512], F32, tag="ppsum")
                        f_sb = spool.tile([128, 512], BF, tag="fsb", bufs=7)
                        for hp in range(8):
                            nc.tensor.matmul(
                                fps[:],
                                wo_sb[:, hp, eoc * 128 : (eoc + 1) * 128],
                                st["oat"][:, hp, :],
                                start=(hp == 0),
                                stop=(hp == 7),
                            )
                        # stores go out on the otherwise-idle GPSIMD (SWDGE)
                        # queue so their copy-waits never clog SP.SEQ, which
                        # carries the transposes and input loads.
                        if final and eoc >= 5:
                            # tail stores ride the (idle) ACT hwdge queue in
                            # natural order right behind their copies
                            nc.scalar.activation(
                                f_sb[:], fps[:], AF.Identity,
                                bias=bo2_sb[:, eoc : eoc + 1], scale=1.0,
                            )
                            nc.scalar.dma_start(out[b, eoc], f_sb[:])
                        else:
                            nc.scalar.activation(
                                f_sb[:], fps[:], AF.Identity,
                                bias=bo2_sb[:, eoc : eoc + 1], scale=1.0,
                            )
                            nc.gpsimd.dma_start(out[b, eoc], f_sb[:])
                    pieces.append(o_piece)
                return pieces

            # Weave plan (keeps every attention phase PE-bound vs the ACT
            # exp stream): att(0): inproj(1); att(1): inproj(2)+op(0)[:4];
            # att(2): inproj(3); att(3): op(0)[4:]+op(1)+op(2) (20 pieces so
            # the final attention phase stays PE-bound too; oat bufs=3).
            st = load_graph(0)
            for p in inproj_pieces(st):
                p()
            states = [st]
            deferred = []
            for b in range(1, BPC):
                st_next = load_graph(b)
                pieces = inproj_pieces(st_next)
                if b == 2:
                    deferred += outproj_pieces(states[0], 0)
                elif b == 3:
                    deferred += outproj_pieces(states[1], 1)
                emit_attention(states[b - 1], pieces)
                states.append(st_next)
            emit_attention(
                states[BPC - 1],
                deferred + outproj_pieces(states[BPC - 2], BPC - 2),
                last=True,
            )
            for p in outproj_pieces(states[BPC - 1], BPC - 1, final=True):
                p()

    nc.compile()
    return nc


def _split_hl(x):
    """fp8 e4m3 hi/lo pair along a new axis 0 (x already pre-scaled)."""
    fp8 = ml_dtypes.float8_e4m3
    xh = x.astype(fp8)
    xl = (x - xh.astype(np.float32)).astype(fp8)
    return np.stack([xh, xl])


def _prep_inputs(query, key, value, attn_mask, in_proj_weight, in_proj_bias,
                 out_proj_weight, out_proj_bias):
    bf16 = ml_dtypes.bfloat16

    def xt_layout(x):  # [L, B, E] -> fp8 hi/lo [B, 128, 2, 8, L]
        xt = np.ascontiguousarray(
            x.reshape(L, B, 8, 128).transpose(1, 3, 2, 0)
        ) * np.float32(XS)
        return np.ascontiguousarray(_split_hl(xt).transpose(1, 2, 0, 3, 4))

    def wt_layout(w):  # [e_out, e_in] -> W^T·WS fp8 hi/lo [128, 2, 8, e_out]
        wt = np.ascontiguousarray(
            w.T.reshape(8, 128, E).transpose(1, 0, 2)
        ) * np.float32(WS)
        return np.ascontiguousarray(_split_hl(wt).transpose(1, 0, 2, 3))

    def wt_bf_layout(w):  # [e_out, e_in] -> W^T as [128, 8, e_out] bf16
        return np.ascontiguousarray(
            w.T.reshape(8, 128, E).transpose(1, 0, 2)
        ).astype(bf16)

    def wt_eo_layout(w):  # [e_out, e_in] -> W^T·WS fp8 hi/lo [eo, 128, 2, 8, 128]
        # wt[p, ei, eo*128 + c] -> arr[eo, p, hl, ei, c]
        wt = w.T.reshape(8, 128, 8, 128)  # [ei, p, eo, c]
        wt = np.ascontiguousarray(wt.transpose(2, 1, 0, 3)) * np.float32(WS)
        return np.ascontiguousarray(_split_hl(wt).transpose(1, 2, 0, 3, 4))

    Wq = in_proj_weight[0:E] / np.float32(np.sqrt(HD))
    Wk = in_proj_weight[E : 2 * E]
    Wv = in_proj_weight[2 * E : 3 * E]
    bq_e = in_proj_bias[0:E] / np.float32(np.sqrt(HD))
    bk_e = in_proj_bias[E : 2 * E]
    bv_e = in_proj_bias[2 * E : 3 * E]

    mneg = np.where(attn_mask, np.float32(0.0), np.float32(1.0))  # [B, q, k]
    # -> [B, k, q] -> [B, 128, 4, q]
    mneg = np.ascontiguousarray(
        mneg.transpose(0, 2, 1).reshape(B, 4, 128, L).transpose(0, 2, 1, 3)
    ).astype(bf16)

    effb = (out_proj_bias + out_proj_weight @ bv_e).astype(np.float32)

    host = {
        "xq": xt_layout(query),
        "xk": xt_layout(key),
        "xv": xt_layout(value),
        "mneg": mneg,
        "wq": wt_eo_layout(Wq),
        "wk": wt_eo_layout(Wk),
        "wv": wt_layout(Wv),
        "wo": wt_bf_layout(out_proj_weight),
        "bq": np.ascontiguousarray(bq_e.reshape(8, 128).T).astype(np.float32),
        "bk": np.ascontiguousarray(bk_e.reshape(8, 128).T).astype(np.float32),
        "bo2": np.ascontiguousarray(effb.reshape(8, 128).T).astype(np.float32),
    }
    shared = {k: host[k] for k in ("wq", "wk", "wv", "wo", "bq", "bk", "bo2")}
    in_maps = []
    for c in range(NCORES):
        sl = slice(c * BPC, (c + 1) * BPC)
        m = dict(shared)
        m["xq"] = np.ascontiguousarray(host["xq"][sl])
        m["xk"] = np.ascontiguousarray(host["xk"][sl])
        m["xv"] = np.ascontiguousarray(host["xv"][sl])
        m["mneg"] = np.ascontiguousarray(host["mneg"][sl])
        in_maps.append(m)
    return in_maps


def kernel(query, key, value, attn_mask, in_proj_weight, in_proj_bias,
           out_proj_weight, out_proj_bias, num_heads, _trace=False):
    query = np.asarray(query, dtype=np.float32)
    key = np.asarray(key, dtype=np.float32)
    value = np.asarray(value, dtype=np.float32)
    attn_mask = np.asarray(attn_mask)
    in_proj_weight = np.asarray(in_proj_weight, dtype=np.float32)
    in_proj_bias = np.asarray(in_proj_bias, dtype=np.float32)
    out_proj_weight = np.asarray(out_proj_weight, dtype=np.float32)
    out_proj_bias = np.asarray(out_proj_bias, dtype=np.float32)
    assert int(num_heads) == H

    if "nc" not in _BUILT:
        _BUILT["nc"] = _build_module()
    nc = _BUILT["nc"]

    in_maps = _prep_inputs(query, key, value, attn_mask, in_proj_weight,
                           in_proj_bias, out_proj_weight, out_proj_bias)
    res = bass_utils.run_bass_kernel_spmd(
        nc, in_maps, core_ids=list(range(NCORES)), trace=_trace
    )
    outs = np.stack([np.asarray(r["out"], dtype=np.float32)
                     for r in res.results])  # [8, BPC, 8, 128, L]
    # full[l, c*BPC+j, eoc*128+p] = outs[c, j, eoc, p, l]
    full = outs.transpose(4, 0, 1, 2, 3).reshape(L, B, E)
    if _trace:
        return np.ascontiguousarray(full.astype(np.float32)), res
    return np.ascontiguousarray(full.astype(np.float32))

